# revision 35
# baseline (speedup 1.0000x reference)
"""Trainium2 Bass kernel for nn_AxialShiftedBlock.

Data-parallel over batch: 32 samples -> 4 per core x 8 cores, weights
replicated, no collectives.

Per-core layout: activations as [channels(partitions), H*W(free)] f32/bf16,
two 128-partition blocks for 256 channels. Spatial 3136 = 7 chunks x 448
(448 = 8 rows of W=56, so chunk edges align with image rows).

Fusions:
  - axial shifts fused into GN1-apply (ACT affine writes shifted views) and
    into conv psum drains (DVE bias-add+cast writes shifted views)
  - channel shuffle + cascade-GN affine folded into host-permuted,
    device-scaled casc weights (shuffle costs zero instructions)
  - residual adds fused into psum drains (scalar_tensor_tensor)
"""
import numpy as np
import ml_dtypes
from contextlib import ExitStack

import concourse.bass as bass
import concourse.mybir as mybir
import concourse.tile as tile
from concourse import bacc
from concourse.bass_utils import run_bass_kernel_spmd

F32 = mybir.dt.float32
BF16 = mybir.dt.bfloat16
AF = mybir.ActivationFunctionType
ALU = mybir.AluOpType

B, C, H, W = 32, 256, 56, 56
HW = H * W            # 3136
NCORES = 8
BL = B // NCORES      # 4 samples per core
NCH = 7               # spatial chunks
CW = HW // NCH        # 448
EPS = 1e-5


def build(bl=BL, gelu_identity=False):
    nc = bacc.Bacc("TRN2")
    gelu_fn = AF.Identity if gelu_identity else AF.Gelu

    x_d = nc.dram_tensor("x", [bl, C, HW], F32, kind="ExternalInput")
    out_d = nc.dram_tensor("out", [bl, C, HW], F32, kind="ExternalOutput")

    c0T_d = nc.dram_tensor("c0T", [64, 64], BF16, kind="ExternalInput")
    c1T_d = nc.dram_tensor("c1T", [96, 96], BF16, kind="ExternalInput")
    c2T_d = nc.dram_tensor("c2T", [112, 112], BF16, kind="ExternalInput")
    zT_d = nc.dram_tensor("zT", [C, C], BF16, kind="ExternalInput")
    fc1T_d = nc.dram_tensor("fc1T", [C, 4 * C], BF16, kind="ExternalInput")
    fc2T_d = nc.dram_tensor("fc2T", [4 * C, C], BF16, kind="ExternalInput")
    cb0_d = nc.dram_tensor("cb0", [64, 1], F32, kind="ExternalInput")
    cb1_d = nc.dram_tensor("cb1", [96, 1], F32, kind="ExternalInput")
    cb2_d = nc.dram_tensor("cb2", [112, 1], F32, kind="ExternalInput")
    # per-block [128,1] vectors packed as [128, nblk]
    n1g_d = nc.dram_tensor("n1g", [128, 2], F32, kind="ExternalInput")
    n1b_d = nc.dram_tensor("n1b", [128, 2], F32, kind="ExternalInput")
    n2g_d = nc.dram_tensor("n2g", [128, 2], F32, kind="ExternalInput")
    n2b_d = nc.dram_tensor("n2b", [128, 2], F32, kind="ExternalInput")
    gz_d = nc.dram_tensor("gz", [128, 2], F32, kind="ExternalInput")
    bz_d = nc.dram_tensor("bz", [128, 2], F32, kind="ExternalInput")
    cascb_d = nc.dram_tensor("cascb", [128, 2], F32, kind="ExternalInput")
    fc1b_d = nc.dram_tensor("fc1b", [128, 8], F32, kind="ExternalInput")
    fc2b_d = nc.dram_tensor("fc2b", [128, 2], F32, kind="ExternalInput")

    with tile.TileContext(nc) as tc, ExitStack() as ctx:
        wpool = ctx.enter_context(tc.tile_pool(name="wpool", bufs=1))
        xpool = ctx.enter_context(tc.tile_pool(name="xpool", bufs=2))
        spool = ctx.enter_context(tc.tile_pool(name="spool", bufs=1))
        zpool = ctx.enter_context(tc.tile_pool(name="zpool", bufs=2))
        hpool = ctx.enter_context(tc.tile_pool(name="hpool", bufs=2))
        vpool = ctx.enter_context(tc.tile_pool(name="vpool", bufs=3))
        wspool = ctx.enter_context(tc.tile_pool(name="wspool", bufs=2))
        pp1 = ctx.enter_context(tc.tile_pool(name="pp1", bufs=2, space="PSUM"))
        ppzo = ctx.enter_context(tc.tile_pool(name="ppzo", bufs=2, space="PSUM"))
        pph = ctx.enter_context(tc.tile_pool(name="pph", bufs=2, space="PSUM"))

        # ---- weights to SBUF (once) ----
        c0T = wpool.tile([64, 64], BF16)
        nc.sync.dma_start(out=c0T, in_=c0T_d[:])
        c1T = wpool.tile([96, 96], BF16)
        nc.sync.dma_start(out=c1T, in_=c1T_d[:])
        c2T = wpool.tile([112, 112], BF16)
        nc.sync.dma_start(out=c2T, in_=c2T_d[:])
        zT0 = wpool.tile([128, C], BF16)
        nc.sync.dma_start(out=zT0, in_=zT_d[0:128])
        zT1 = wpool.tile([128, C], BF16)
        nc.sync.dma_start(out=zT1, in_=zT_d[128:256])
        fc1T0 = wpool.tile([128, 4 * C], BF16)
        nc.sync.dma_start(out=fc1T0, in_=fc1T_d[0:128])
        fc1T1 = wpool.tile([128, 4 * C], BF16)
        nc.sync.dma_start(out=fc1T1, in_=fc1T_d[128:256])
        fc2T = wpool.tile([128, 8, C], BF16)
        nc.sync.dma_start(out=fc2T, in_=fc2T_d[:].rearrange("(k p) m -> p k m", p=128))
        cb0 = wpool.tile([64, 1], F32)
        nc.sync.dma_start(out=cb0, in_=cb0_d[:])
        cb1 = wpool.tile([96, 1], F32)
        nc.sync.dma_start(out=cb1, in_=cb1_d[:])
        cb2 = wpool.tile([112, 1], F32)
        nc.sync.dma_start(out=cb2, in_=cb2_d[:])
        vecs = {}
        for nm, d in [("n1g", n1g_d), ("n1b", n1b_d), ("n2g", n2g_d),
                      ("n2b", n2b_d), ("gz", gz_d), ("bz", bz_d),
                      ("cascb", cascb_d), ("fc1b", fc1b_d), ("fc2b", fc2b_d)]:
            t = wpool.tile(list(d.shape), F32, name=f"v_{nm}")
            nc.sync.dma_start(out=t, in_=d[:])
            vecs[nm] = t
        ones_f = wpool.tile([128, 128], F32)
        nc.vector.memset(ones_f, 1.0)
        eps_t = wpool.tile([128, 1], F32)
        nc.vector.memset(eps_t, EPS)
        # GN1-fold helper vectors: per-conv input-channel gamma/beta at the
        # conv's partition base (zeros on recurrent-input rows), plus a mask
        # that is 1.0 on recurrent rows (their weight scale must be 1).
        g_c0 = wpool.tile([64, 1], F32)
        nc.sync.dma_start(out=g_c0, in_=n1g_d[64:128, 0:1])
        b_c0 = wpool.tile([64, 1], F32)
        nc.sync.dma_start(out=b_c0, in_=n1b_d[64:128, 0:1])
        m_c0 = wpool.tile([64, 1], F32)
        nc.vector.memset(m_c0, 0.0)
        g_c1 = wpool.tile([96, 1], F32)
        nc.vector.memset(g_c1, 0.0)
        nc.sync.dma_start(out=g_c1[0:64], in_=n1g_d[0:64, 1:2])
        b_c1 = wpool.tile([96, 1], F32)
        nc.vector.memset(b_c1, 0.0)
        nc.sync.dma_start(out=b_c1[0:64], in_=n1b_d[0:64, 1:2])
        m_c1 = wpool.tile([96, 1], F32)
        nc.vector.memset(m_c1, 0.0)
        nc.vector.memset(m_c1[64:96], 1.0)
        g_c2 = wpool.tile([112, 1], F32)
        nc.vector.memset(g_c2, 0.0)
        nc.sync.dma_start(out=g_c2[0:64], in_=n1g_d[64:128, 1:2])
        b_c2 = wpool.tile([112, 1], F32)
        nc.vector.memset(b_c2, 0.0)
        nc.sync.dma_start(out=b_c2[0:64], in_=n1b_d[64:128, 1:2])
        m_c2 = wpool.tile([112, 1], F32)
        nc.vector.memset(m_c2, 0.0)
        nc.vector.memset(m_c2[64:112], 1.0)
        conv_fold = [(c0T, g_c0, b_c0, m_c0, cb0, 64, 64),
                     (c1T, g_c1, b_c1, m_c1, cb1, 96, 96),
                     (c2T, g_c2, b_c2, m_c2, cb2, 112, 112)]

        def cols(n):
            return slice(n * CW, (n + 1) * CW)

        def gn_scale_bias(Xs, g_sl, b_sl):
            """GroupNorm(1,C) stats over two [128,HW] blocks -> per-channel
            (scale, bias) [128,1] f32 tiles for each block."""
            ps = pp1.tile([128, 3], F32, tag="convps", name="ps_stat")
            for k, X in enumerate(Xs):
                stats = vpool.tile([128, NCH, 6], F32, tag="stats", name="stats")
                for j in range(NCH):
                    nc.vector.bn_stats(out=stats[:, j, :], in_=X[:, cols(j)])
                mv = vpool.tile([128, 2], F32, tag="mv", name="mv")
                nc.vector.bn_aggr(out=mv, in_=stats)
                s3 = vpool.tile([128, 3], F32, tag="s3", name="s3")
                nc.vector.tensor_copy(out=s3[:, 0:2], in_=mv)
                nc.vector.tensor_mul(s3[:, 2:3], mv[:, 0:1], mv[:, 0:1])
                nc.tensor.matmul(out=ps, lhsT=ones_f, rhs=s3,
                                 start=(k == 0), stop=(k == 1))
            pssb = vpool.tile([128, 3], F32, tag="pssb", name="pssb")
            nc.vector.tensor_copy(out=pssb, in_=ps)
            mean_t = vpool.tile([128, 1], F32, tag="mean_t", name="mean_t")
            nc.vector.tensor_scalar_mul(mean_t, pssb[:, 0:1], 1.0 / C)
            msq = vpool.tile([128, 1], F32, tag="msq", name="msq")
            nc.vector.tensor_mul(msq, mean_t, mean_t)
            e2 = vpool.tile([128, 1], F32, tag="e2", name="e2")
            nc.vector.tensor_add(e2, pssb[:, 1:2], pssb[:, 2:3])
            var_t = vpool.tile([128, 1], F32, tag="var_t", name="var_t")
            nc.vector.tensor_scalar(out=var_t, in0=e2, scalar1=1.0 / C,
                                    scalar2=msq, op0=ALU.mult, op1=ALU.subtract)
            sd = vpool.tile([128, 1], F32, tag="sd", name="sd")
            nc.scalar.activation(out=sd, in_=var_t, func=AF.Sqrt, bias=eps_t)
            rstd = vpool.tile([128, 1], F32, tag="rstd", name="rstd")
            nc.vector.reciprocal(rstd, sd)
            nms = vpool.tile([128, 1], F32, tag="nms", name="nms")
            nc.vector.tensor_scalar(out=nms, in0=mean_t, scalar1=rstd,
                                    scalar2=-1.0, op0=ALU.mult, op1=ALU.mult)
            outs = []
            for k in range(2):
                sc_ = vpool.tile([128, 1], F32, tag=f"gsc{k}", name=f"gsc{k}")
                nc.vector.tensor_mul(sc_, g_sl[k], rstd)
                bi_ = vpool.tile([128, 1], F32, tag=f"gbi{k}", name=f"gbi{k}")
                nc.vector.scalar_tensor_tensor(out=bi_, in0=g_sl[k], scalar=nms,
                                               in1=b_sl[k], op0=ALU.mult,
                                               op1=ALU.add)
                outs.append((sc_, bi_))
            return outs, rstd, nms

        def aff(dst, src, sc, bi):
            # GN affine on DVE (2x mode) -- frees ACT for gelu
            nc.vector.tensor_scalar(out=dst, in0=src, scalar1=sc, scalar2=bi,
                                    op0=ALU.mult, op1=ALU.add)

        def shift_dma(dst, d0, src_t, s0, s1, shift, eng="dma"):
            """Copy src rows [s0:s1] into dst rows [d0:d0+n] with the axial
            shift (edge keeps original). eng='dve' only for pieces whose
            in/out partition bases satisfy the 0/32/64/96 alignment rule;
            DMA is exempt from that constraint."""
            n = s1 - s0
            d1 = d0 + n

            def cp(dsl, ssl):
                if eng == "dve":
                    nc.vector.tensor_copy(out=dst[d0:d1, dsl],
                                          in_=src_t[s0:s1, ssl])
                else:
                    nc.sync.dma_start(out=dst[d0:d1, dsl], in_=src_t[s0:s1, ssl])

            def cp_edge(d3, s3_):
                if eng == "dve":
                    nc.vector.tensor_copy(out=d3, in_=s3_)
                else:
                    nc.sync.dma_start(out=d3, in_=s3_)

            if shift == "N":
                cp(slice(0, HW), slice(0, HW))
            elif shift == "R":
                cp(slice(1, HW), slice(0, HW - 1))
                d3 = dst[d0:d1].rearrange("p (h w) -> p h w", w=W)
                s3_ = src_t[s0:s1].rearrange("p (h w) -> p h w", w=W)
                cp_edge(d3[:, :, 0:1], s3_[:, :, 0:1])
            elif shift == "L":
                cp(slice(0, HW - 1), slice(1, HW))
                d3 = dst[d0:d1].rearrange("p (h w) -> p h w", w=W)
                s3_ = src_t[s0:s1].rearrange("p (h w) -> p h w", w=W)
                cp_edge(d3[:, :, W - 1:W], s3_[:, :, W - 1:W])
            elif shift == "D":
                cp(slice(W, HW), slice(0, HW - W))
                cp(slice(0, W), slice(0, W))
            elif shift == "U":
                cp(slice(0, HW - W), slice(W, HW))
                cp(slice(HW - W, HW), slice(HW - W, HW))

        def aff_act(dst, src, sc, bi):
            # GN affine on ACT (idle during the cascade phase)
            nc.scalar.activation(out=dst, in_=src, func=AF.Identity,
                                 bias=bi, scale=sc)

        def gn_finish(ps, g_sl, b_sl, inv):
            """From psum [128,2] = [sum(x), sum(x^2)] broadcast over
            partitions, compute per-block (scale, bias)."""
            pssb = vpool.tile([128, 2], F32, tag="pssb", name="pssb")
            nc.vector.tensor_copy(out=pssb, in_=ps)
            mean_t = vpool.tile([128, 1], F32, tag="mean_t", name="mean_t")
            nc.vector.tensor_scalar_mul(mean_t, pssb[:, 0:1], inv)
            msq = vpool.tile([128, 1], F32, tag="msq", name="msq")
            nc.vector.tensor_mul(msq, mean_t, mean_t)
            var_t = vpool.tile([128, 1], F32, tag="var_t", name="var_t")
            nc.vector.tensor_scalar(out=var_t, in0=pssb[:, 1:2], scalar1=inv,
                                    scalar2=msq, op0=ALU.mult, op1=ALU.subtract)
            sd = vpool.tile([128, 1], F32, tag="sd", name="sd")
            nc.scalar.activation(out=sd, in_=var_t, func=AF.Sqrt, bias=eps_t)
            rstd = vpool.tile([128, 1], F32, tag="rstd", name="rstd")
            nc.vector.reciprocal(rstd, sd)
            nms = vpool.tile([128, 1], F32, tag="nms", name="nms")
            nc.vector.tensor_scalar(out=nms, in0=mean_t, scalar1=rstd,
                                    scalar2=-1.0, op0=ALU.mult, op1=ALU.mult)
            outs = []
            for k in range(2):
                sc_ = vpool.tile([128, 1], F32, tag=f"gsc{k}", name=f"gsc{k}")
                nc.vector.tensor_mul(sc_, g_sl[k], rstd)
                bi_ = vpool.tile([128, 1], F32, tag=f"gbi{k}", name=f"gbi{k}")
                nc.vector.scalar_tensor_tensor(out=bi_, in0=g_sl[k], scalar=nms,
                                               in1=b_sl[k], op0=ALU.mult,
                                               op1=ALU.add)
                outs.append((sc_, bi_))
            return outs, rstd, nms

        def gn_scale_bias_pool(Xs, g_sl, b_sl):
            """GroupNorm stats via GPSIMD (idle engine): per-channel sum and
            sum-of-squares, then cross-partition ones-matmul."""
            ps = pp1.tile([128, 2], F32, tag="convps", name="ps_stat2")
            for k, X in enumerate(Xs):
                r2 = vpool.tile([128, 2], F32, tag="r2", name="r2")
                scr = spool.tile([128, HW], BF16, tag="scr", name="scr")
                nc.gpsimd.tensor_scalar(out=scr, in0=X, scalar1=1.0,
                                        scalar2=None, op0=ALU.mult,
                                        op1=ALU.add, accum_out=r2[:, 0:1])
                scr2 = spool.tile([128, HW], BF16, tag="scr", name="scr2")
                nc.gpsimd.scalar_tensor_tensor(
                    out=scr2, in0=X, scalar=0.0, in1=X,
                    op0=ALU.add, op1=ALU.mult, accum_out=r2[:, 1:2])
                nc.tensor.matmul(out=ps, lhsT=ones_f, rhs=r2,
                                 start=(k == 0), stop=(k == 1))
            return gn_finish(ps, g_sl, b_sl, 1.0 / (C * HW))

        # ---------------- per-sample pipeline ----------------
        # stage A: load, GN1, shifts, cascade convs, casc GN+conv+residual
        # stage B: GN2, fc1+gelu+fc2, output
        # Every engine executes its instruction stream in (priority ~)
        # emission order, so coarse per-sample emission serializes samples.
        # Emit in fine-grained interleave: parts of B(b) alternate with
        # parts of A(b+1), letting b+1's (DVE-heavy) cascade fill the gaps
        # of b's (PE/ACT-heavy) fc phase on every engine.
        def stage_a_parts(b, st):
            def p0():
                st["XA"] = xpool.tile([128, HW], F32, tag="XA", name="XA")
                st["XB"] = xpool.tile([128, HW], F32, tag="XB", name="XB")
                nc.sync.dma_start(out=st["XA"], in_=x_d[b, 0:128, :])
                nc.scalar.dma_start(out=st["XB"], in_=x_d[b, 128:256, :])
                st["gn1"], st["rstd1"], st["nms1"] = gn_scale_bias(
                    [st["XA"], st["XB"]],
                    [vecs["n1g"][:, 0:1], vecs["n1g"][:, 1:2]],
                    [vecs["n1b"][:, 0:1], vecs["n1b"][:, 1:2]])
                # fold GN1 into conv weights: scaled weights + bias vectors
                st["cTs"], st["cbf"] = [], []
                for i, (cT, g_c, b_c, m_c, cb_c, Ki, Mi) in enumerate(conv_fold):
                    scv = vpool.tile([Ki, 1], F32, tag=f"scv{i}", name=f"scv{i}")
                    nc.vector.scalar_tensor_tensor(
                        out=scv, in0=g_c, scalar=st["rstd1"][0:Ki], in1=m_c,
                        op0=ALU.mult, op1=ALU.add)
                    bvv = vpool.tile([Ki, 1], F32, tag=f"bvv{i}", name=f"bvv{i}")
                    nc.vector.scalar_tensor_tensor(
                        out=bvv, in0=g_c, scalar=st["nms1"][0:Ki], in1=b_c,
                        op0=ALU.mult, op1=ALU.add)
                    bvvb = vpool.tile([Ki, 1], BF16, tag=f"bvvb{i}",
                                      name=f"bvvb{i}")
                    nc.vector.tensor_copy(out=bvvb, in_=bvv)
                    cTs = wspool.tile([Ki, Mi], BF16, tag=f"cts{i}",
                                      name=f"cts{i}")
                    nc.vector.tensor_scalar_mul(cTs, cT, scv)
                    psbf = pp1.tile([Mi, 1], F32, tag="convps", name="psbfc")
                    nc.tensor.matmul(out=psbf, lhsT=cT, rhs=bvvb,
                                     start=True, stop=True)
                    bfc = wspool.tile([Mi, 1], F32, tag=f"bfc{i}",
                                      name=f"bfc{i}")
                    nc.scalar.activation(out=bfc, in_=psbf, func=AF.Identity,
                                         bias=cb_c)
                    st["cTs"].append(cTs)
                    st["cbf"].append(bfc)

            def p1():
                XA, XB = st["XA"], st["XB"]
                (scA, biA), (scB, biB) = st["gn1"]
                S1 = st["S1"] = spool.tile([64, HW], BF16, tag="sh_d", name="S1")
                S2 = st["S2"] = spool.tile([96, HW], BF16, tag="sh_e", name="S2")
                S3 = st["S3"] = spool.tile([112, HW], BF16, tag="sh_f", name="S3")
                st["Z0"] = zpool.tile([128, HW], BF16, tag="Z0", name="Z0")
                Z1 = st["Z1"] = zpool.tile([128, HW], BF16, tag="Z1", name="Z1")
                Y1 = spool.tile([64, HW], BF16, tag="sh_a", name="Y1")
                Y2 = spool.tile([64, HW], BF16, tag="sh_b", name="Y2")
                Y3 = spool.tile([64, HW], BF16, tag="sh_c", name="Y3")
                # xs0 needs GN1 applied (it feeds the cascade GN directly);
                # xs1/2/3 are consumed by convs whose weights absorb GN1,
                # so they are plain bf16 casts with no stats dependency.
                aff_act(Z1[64:128], XA[0:64], scA[0:64], biA[0:64])   # xs0
                nc.scalar.copy(out=Y1, in_=XA[64:128])                # xs1 raw
                nc.vector.tensor_copy(out=Y2, in_=XB[0:64])           # xs2 raw
                nc.vector.tensor_copy(out=Y3, in_=XB[64:128])         # xs3 raw
                shift_dma(S1, 0, Y1, 0, 16, "R", eng="dve")
                shift_dma(S1, 16, Y1, 16, 32, "L")
                shift_dma(S1, 32, Y1, 32, 48, "D", eng="dve")
                shift_dma(S1, 48, Y1, 48, 64, "U")
                shift_dma(S2, 0, Y2, 0, 24, "R", eng="dve")
                shift_dma(S2, 24, Y2, 24, 48, "L")
                shift_dma(S2, 48, Y2, 48, 64, "D")
                shift_dma(S3, 0, Y3, 0, 28, "R", eng="dve")
                shift_dma(S3, 28, Y3, 28, 56, "L")
                shift_dma(S3, 56, Y3, 56, 64, "D")

            def p2():
                # conv c0: [64]->[64] = [x1(32) | o1(32)]; o1 direct to Z0
                S1, S2, Z0 = st["S1"], st["S2"], st["Z0"]
                X1 = spool.tile([32, HW], BF16, tag="sh_a", name="X1")
                for n in range(NCH):
                    ps0 = pp1.tile([64, CW], F32, tag="convps", name="ps0")
                    nc.tensor.matmul(out=ps0, lhsT=st["cTs"][0], rhs=S1[:, cols(n)],
                                     start=True, stop=True)
                    nc.vector.tensor_scalar_add(out=X1[:, cols(n)],
                                                in0=ps0[0:32],
                                                scalar1=st["cbf"][0][0:32])
                    nc.vector.tensor_scalar_add(out=Z0[0:32, cols(n)],
                                                in0=ps0[32:64],
                                                scalar1=st["cbf"][0][32:64])
                shift_dma(S2, 64, X1, 0, 8, "D", eng="dve")   # x1 q2-part
                shift_dma(S2, 72, X1, 8, 32, "U")             # x1 q3

            def p3():
                # conv c1: [96]->[96] = [x1b(48) | o2(48)]
                S2, S3, Z0 = st["S2"], st["S3"], st["Z0"]
                O2 = spool.tile([96, HW], BF16, tag="sh_d", name="O2")
                for n in range(NCH):
                    ps1 = pp1.tile([96, CW], F32, tag="convps", name="ps1")
                    nc.tensor.matmul(out=ps1, lhsT=st["cTs"][1], rhs=S2[:, cols(n)],
                                     start=True, stop=True)
                    nc.vector.tensor_scalar_add(out=O2[:, cols(n)], in0=ps1,
                                                scalar1=st["cbf"][1])
                shift_dma(S3, 64, O2, 0, 20, "D", eng="dve")  # x1b q2-part
                shift_dma(S3, 84, O2, 20, 48, "U")            # x1b q3
                shift_dma(Z0, 32, O2, 48, 96, "N")            # o2

            def p4():
                # conv c2: [112]->[112], rows [o3b(64) | o3a(48)]
                S3, Z0, Z1 = st["S3"], st["Z0"], st["Z1"]
                O3 = spool.tile([48, HW], BF16, tag="sh_e", name="O3")
                for n in range(NCH):
                    ps2 = pp1.tile([112, CW], F32, tag="convps", name="ps2")
                    nc.tensor.matmul(out=ps2, lhsT=st["cTs"][2], rhs=S3[:, cols(n)],
                                     start=True, stop=True)
                    nc.vector.tensor_scalar_add(out=Z1[0:64, cols(n)],
                                                in0=ps2[0:64],
                                                scalar1=st["cbf"][2][0:64])
                    nc.vector.tensor_scalar_add(out=O3[:, cols(n)],
                                                in0=ps2[64:112],
                                                scalar1=st["cbf"][2][64:112])
                shift_dma(Z0, 80, O3, 0, 48, "N")             # o3a

            def p5():
                # cascade GN (folded into weights): stats + scaled weights
                Z0, Z1 = st["Z0"], st["Z1"]
                gnz, _, _ = gn_scale_bias(
                    [Z0, Z1],
                    [vecs["gz"][:, 0:1], vecs["gz"][:, 1:2]],
                    [vecs["bz"][:, 0:1], vecs["bz"][:, 1:2]])
                Wzs, bzbf = [], []
                for k, (zT_k, (sc_k, bi_k)) in enumerate(zip([zT0, zT1], gnz)):
                    w_s = wspool.tile([128, C], BF16, tag=f"wzs{k}",
                                      name=f"wzs{k}")
                    nc.vector.tensor_scalar_mul(w_s, zT_k, sc_k)
                    Wzs.append(w_s)
                    bb = wspool.tile([128, 1], BF16, tag=f"bzbf{k}",
                                     name=f"bzbf{k}")
                    nc.vector.tensor_copy(out=bb, in_=bi_k)
                    bzbf.append(bb)
                st["Wzs"] = Wzs
                bfs = []
                for m in range(2):
                    psbf = pp1.tile([128, 1], F32, tag="convps", name="psbf")
                    for k in range(2):
                        nc.tensor.matmul(
                            out=psbf,
                            lhsT=[zT0, zT1][k][:, 128 * m:128 * m + 128],
                            rhs=bzbf[k], start=(k == 0), stop=(k == 1))
                    bf_sb = wspool.tile([128, 1], F32, tag=f"bfsb{m}",
                                        name=f"bfsb{m}")
                    nc.scalar.activation(out=bf_sb, in_=psbf, func=AF.Identity,
                                         bias=vecs["cascb"][:, m:m + 1])
                    bfs.append(bf_sb)
                st["bfs"] = bfs

            def casc_chunks(lo, hi):
                def run():
                    Z0, Z1 = st["Z0"], st["Z1"]
                    Wzs, bfs = st["Wzs"], st["bfs"]
                    for n in range(lo, hi):
                        for m, X_m in enumerate([st["XA"], st["XB"]]):
                            psz = ppzo.tile([128, CW], F32, tag="pszo",
                                            name="psz")
                            nc.tensor.matmul(
                                out=psz, lhsT=Wzs[0][:, 128 * m:128 * m + 128],
                                rhs=Z0[:, cols(n)], start=True, stop=False)
                            nc.tensor.matmul(
                                out=psz, lhsT=Wzs[1][:, 128 * m:128 * m + 128],
                                rhs=Z1[:, cols(n)], start=False, stop=True)
                            # x_new = x + psz + bf  (in place into X_m)
                            nc.vector.scalar_tensor_tensor(
                                out=X_m[:, cols(n)], in0=psz, scalar=bfs[m],
                                in1=X_m[:, cols(n)], op0=ALU.add, op1=ALU.add)
                return run

            return [p0, p1, p2, p3, p4, p5,
                    casc_chunks(0, 2), casc_chunks(2, 5), casc_chunks(5, NCH)]

        def stage_b_parts(b, st):
            def q0():
                XA, XB = st["XA"], st["XB"]
                gn2, _, _ = gn_scale_bias(
                    [XA, XB],
                    [vecs["n2g"][:, 0:1], vecs["n2g"][:, 1:2]],
                    [vecs["n2b"][:, 0:1], vecs["n2b"][:, 1:2]])
                M0 = st["M0"] = zpool.tile([128, HW], BF16, tag="M0", name="M0")
                M1 = st["M1"] = zpool.tile([128, HW], BF16, tag="M1", name="M1")
                # GN2 folded into fc1 weights: M tiles are raw bf16 casts
                nc.vector.tensor_copy(out=M0, in_=XA)
                nc.scalar.copy(out=M1, in_=XB)
                f1s, bv2 = [], []
                for k, fT in enumerate([fc1T0, fc1T1]):
                    sck, bik = gn2[k]
                    fs = wspool.tile([128, 4 * C], BF16, tag=f"f1s{k}",
                                     name=f"f1s{k}")
                    nc.vector.tensor_scalar_mul(fs, fT, sck)
                    f1s.append(fs)
                    bb = vpool.tile([128, 1], BF16, tag=f"bv2{k}",
                                    name=f"bv2{k}")
                    nc.vector.tensor_copy(out=bb, in_=bik)
                    bv2.append(bb)
                psb1 = pp1.tile([128, 8], F32, tag="convps", name="psb1")
                for mo in range(8):
                    for k in range(2):
                        nc.tensor.matmul(
                            out=psb1[:, mo:mo + 1],
                            lhsT=[fc1T0, fc1T1][k][:, 128 * mo:128 * mo + 128],
                            rhs=bv2[k], start=(k == 0), stop=(k == 1))
                gb1 = vpool.tile([128, 8], F32, tag="gb1", name="gb1")
                nc.vector.tensor_add(gb1, psb1, vecs["fc1b"])
                st["f1s"], st["gb1"] = f1s, gb1

            def fc_pair(n0):
                def run():
                    XA, XB, M0, M1 = st["XA"], st["XB"], st["M0"], st["M1"]
                    npair = min(2, NCH - n0)
                    pw = npair * CW
                    h = hpool.tile([128, 8, 2, CW], BF16, tag="h", name="h")
                    for mo in range(8):
                        psh = pph.tile([128, 2, 512], F32, tag="psh",
                                       name="psh")
                        for j in range(npair):
                            nc.tensor.matmul(
                                out=psh[:, j, 0:CW],
                                lhsT=st["f1s"][0][:, 128 * mo:128 * mo + 128],
                                rhs=M0[:, cols(n0 + j)], start=True, stop=False)
                            nc.tensor.matmul(
                                out=psh[:, j, 0:CW],
                                lhsT=st["f1s"][1][:, 128 * mo:128 * mo + 128],
                                rhs=M1[:, cols(n0 + j)], start=False, stop=True)
                        nc.scalar.activation(out=h[:, mo, 0:npair, :],
                                             in_=psh[:, 0:npair, 0:CW],
                                             func=gelu_fn,
                                             bias=st["gb1"][:, mo:mo + 1])
                    for j in range(npair):
                        n = n0 + j
                        for m, X_m in enumerate([XA, XB]):
                            pso = ppzo.tile([128, CW], F32, tag="pszo",
                                            name="pso")
                            for k in range(8):
                                nc.tensor.matmul(
                                    out=pso,
                                    lhsT=fc2T[:, k, 128 * m:128 * m + 128],
                                    rhs=h[:, k, j, :],
                                    start=(k == 0), stop=(k == 7))
                            # out = x_new + pso + fc2_b, in place into X_m
                            nc.vector.scalar_tensor_tensor(
                                out=X_m[:, cols(n)], in0=pso,
                                scalar=vecs["fc2b"][:, m:m + 1],
                                in1=X_m[:, cols(n)], op0=ALU.add, op1=ALU.add)
                    pc = slice(n0 * CW, (n0 + npair) * CW)
                    nc.sync.dma_start(out=out_d[b, 0:128, pc],
                                      in_=st["XA"][:, pc])
                    nc.scalar.dma_start(out=out_d[b, 128:256, pc],
                                        in_=st["XB"][:, pc])
                return run

            return [q0] + [fc_pair(n0) for n0 in range(0, NCH, 2)]

        states = [dict() for _ in range(bl)]
        for part in stage_a_parts(0, states[0]):
            part()
        for b in range(bl):
            bq = stage_b_parts(b, states[b])
            ap = stage_a_parts(b + 1, states[b + 1]) if b + 1 < bl else []
            # round-robin interleave
            i = j = 0
            while i < len(bq) or j < len(ap):
                if i < len(bq):
                    bq[i](); i += 1
                if j < len(ap):
                    ap[j](); j += 1

    nc.compile()
    return nc


def prep_inputs(c0_w, c0_b, c1_w, c1_b, c2_w, c2_b, casc_w, casc_b, casc_g,
                casc_beta, n1_g, n1_b, n2_g, n2_b, fc1_w, fc1_b, fc2_w, fc2_b):
    """Host-side weight prep: transposes, bf16 casts, shuffle permutation."""
    bf = lambda a: np.ascontiguousarray(a).astype(ml_dtypes.bfloat16)
    f32 = lambda a: np.ascontiguousarray(a, dtype=np.float32)

    # channel shuffle (groups=2): shuffled[2i+j] = z[j*128+i]
    zc = np.arange(C)
    sc_of_zc = 2 * (zc % 128) + (zc // 128)

    def blocks(v):  # [256] -> [128, 2]
        return f32(np.stack([v[0:128], v[128:256]], axis=1))

    # c2 output rows reordered to [o3b(ch 48:112) | o3a(ch 0:48)] so o3b
    # drains partition-aligned straight into Z1[0:64]
    c2perm = np.concatenate([np.arange(48, 112), np.arange(0, 48)])
    return {
        "c0T": bf(c0_w.T), "c1T": bf(c1_w.T), "c2T": bf(c2_w[c2perm].T),
        "zT": bf(casc_w[:, sc_of_zc].T),
        "fc1T": bf(fc1_w.T), "fc2T": bf(fc2_w.T),
        "cb0": f32(c0_b[:, None]), "cb1": f32(c1_b[:, None]),
        "cb2": f32(c2_b[c2perm][:, None]),
        "n1g": blocks(n1_g), "n1b": blocks(n1_b),
        "n2g": blocks(n2_g), "n2b": blocks(n2_b),
        "gz": blocks(casc_g[sc_of_zc]), "bz": blocks(casc_beta[sc_of_zc]),
        "cascb": blocks(casc_b),
        "fc1b": f32(fc1_b.reshape(8, 128).T),
        "fc2b": blocks(fc2_b),
    }


_NC_CACHE = {}


def kernel(**inputs):
    x = np.asarray(inputs["x"], dtype=np.float32)
    wmap = prep_inputs(**{k: np.asarray(v) for k, v in inputs.items()
                          if k != "x"})

    key = "main"
    if key not in _NC_CACHE:
        _NC_CACHE[key] = build()
    nc = _NC_CACHE[key]

    xr = x.reshape(B, C, HW)
    in_maps = []
    for c in range(NCORES):
        m = dict(wmap)
        m["x"] = np.ascontiguousarray(xr[c * BL:(c + 1) * BL])
        in_maps.append(m)

    res = run_bass_kernel_spmd(nc, in_maps, core_ids=list(range(NCORES)))
    out = np.concatenate([r["out"] for r in res.results], axis=0)
    return out.reshape(B, C, H, W).astype(np.float32)


if __name__ == "__main__":
    rng = np.random.default_rng(0)
    fake = {
        "x": rng.standard_normal((B, C, H, W), dtype=np.float32),
        "c0_w": rng.standard_normal((64, 64), dtype=np.float32) * 0.02,
        "c0_b": np.zeros(64, np.float32),
        "c1_w": rng.standard_normal((96, 96), dtype=np.float32) * 0.02,
        "c1_b": np.zeros(96, np.float32),
        "c2_w": rng.standard_normal((112, 112), dtype=np.float32) * 0.02,
        "c2_b": np.zeros(112, np.float32),
        "casc_w": rng.standard_normal((C, C), dtype=np.float32) * 0.02,
        "casc_b": np.zeros(C, np.float32),
        "casc_g": np.ones(C, np.float32), "casc_beta": np.zeros(C, np.float32),
        "n1_g": np.ones(C, np.float32), "n1_b": np.zeros(C, np.float32),
        "n2_g": np.ones(C, np.float32), "n2_b": np.zeros(C, np.float32),
        "fc1_w": rng.standard_normal((4 * C, C), dtype=np.float32) * 0.02,
        "fc1_b": np.zeros(4 * C, np.float32),
        "fc2_w": rng.standard_normal((C, 4 * C), dtype=np.float32) * 0.02,
        "fc2_b": np.zeros(C, np.float32),
    }
    out = kernel(**fake)
    print("kernel ran, out shape", out.shape)


# revision 36
# speedup vs baseline: 1.1810x; 1.1810x over previous
"""Trainium2 Bass kernel for nn_AxialShiftedBlock.

Data-parallel over batch: 32 samples -> 4 per core x 8 cores, weights
replicated, no collectives.

Per-core layout: activations as [channels(partitions), H*W(free)] f32/bf16,
two 128-partition blocks for 256 channels. Spatial 3136 = 7 chunks x 448
(448 = 8 rows of W=56, so chunk edges align with image rows).

Fusions:
  - axial shifts fused into GN1-apply (ACT affine writes shifted views) and
    into conv psum drains (DVE bias-add+cast writes shifted views)
  - channel shuffle + cascade-GN affine folded into host-permuted,
    device-scaled casc weights (shuffle costs zero instructions)
  - residual adds fused into psum drains (scalar_tensor_tensor)
"""
import numpy as np
import ml_dtypes
from contextlib import ExitStack

import concourse.bass as bass
import concourse.mybir as mybir
import concourse.tile as tile
from concourse import bacc
from concourse.bass_utils import run_bass_kernel_spmd

F32 = mybir.dt.float32
BF16 = mybir.dt.bfloat16
AF = mybir.ActivationFunctionType
ALU = mybir.AluOpType

B, C, H, W = 32, 256, 56, 56
HW = H * W            # 3136
NCORES = 8
BL = B // NCORES      # 4 samples per core
NCH = 7               # spatial chunks
CW = HW // NCH        # 448
EPS = 1e-5


def build(bl=BL, gelu_identity=False):
    nc = bacc.Bacc("TRN2")
    gelu_fn = AF.Identity if gelu_identity else AF.Gelu

    x_d = nc.dram_tensor("x", [bl, C, HW], F32, kind="ExternalInput")
    out_d = nc.dram_tensor("out", [bl, C, HW], F32, kind="ExternalOutput")

    c0T_d = nc.dram_tensor("c0T", [64, 64], BF16, kind="ExternalInput")
    c1T_d = nc.dram_tensor("c1T", [96, 96], BF16, kind="ExternalInput")
    c2T_d = nc.dram_tensor("c2T", [112, 112], BF16, kind="ExternalInput")
    zT_d = nc.dram_tensor("zT", [C, C], BF16, kind="ExternalInput")
    fc1T_d = nc.dram_tensor("fc1T", [C, 4 * C], BF16, kind="ExternalInput")
    fc2T_d = nc.dram_tensor("fc2T", [4 * C, C], BF16, kind="ExternalInput")
    cb0_d = nc.dram_tensor("cb0", [64, 1], F32, kind="ExternalInput")
    cb1_d = nc.dram_tensor("cb1", [96, 1], F32, kind="ExternalInput")
    cb2_d = nc.dram_tensor("cb2", [112, 1], F32, kind="ExternalInput")
    # per-block [128,1] vectors packed as [128, nblk]
    n1g_d = nc.dram_tensor("n1g", [128, 2], F32, kind="ExternalInput")
    n1b_d = nc.dram_tensor("n1b", [128, 2], F32, kind="ExternalInput")
    n2g_d = nc.dram_tensor("n2g", [128, 2], F32, kind="ExternalInput")
    n2b_d = nc.dram_tensor("n2b", [128, 2], F32, kind="ExternalInput")
    gz_d = nc.dram_tensor("gz", [128, 2], F32, kind="ExternalInput")
    bz_d = nc.dram_tensor("bz", [128, 2], F32, kind="ExternalInput")
    cascb_d = nc.dram_tensor("cascb", [128, 2], F32, kind="ExternalInput")
    fc1b_d = nc.dram_tensor("fc1b", [128, 8], F32, kind="ExternalInput")
    fc2b_d = nc.dram_tensor("fc2b", [128, 2], F32, kind="ExternalInput")

    with tile.TileContext(nc) as tc, ExitStack() as ctx:
        wpool = ctx.enter_context(tc.tile_pool(name="wpool", bufs=1))
        xpool = ctx.enter_context(tc.tile_pool(name="xpool", bufs=2))
        spool = ctx.enter_context(tc.tile_pool(name="spool", bufs=1))
        zpool = ctx.enter_context(tc.tile_pool(name="zpool", bufs=2))
        hpool = ctx.enter_context(tc.tile_pool(name="hpool", bufs=2))
        vpool = ctx.enter_context(tc.tile_pool(name="vpool", bufs=3))
        wspool = ctx.enter_context(tc.tile_pool(name="wspool", bufs=2))
        pp1 = ctx.enter_context(tc.tile_pool(name="pp1", bufs=2, space="PSUM"))
        ppzo = ctx.enter_context(tc.tile_pool(name="ppzo", bufs=2, space="PSUM"))
        pph = ctx.enter_context(tc.tile_pool(name="pph", bufs=2, space="PSUM"))

        # ---- weights to SBUF (once) ----
        c0T = wpool.tile([64, 64], BF16)
        nc.sync.dma_start(out=c0T, in_=c0T_d[:])
        c1T = wpool.tile([96, 96], BF16)
        nc.sync.dma_start(out=c1T, in_=c1T_d[:])
        c2T = wpool.tile([112, 112], BF16)
        nc.sync.dma_start(out=c2T, in_=c2T_d[:])
        zT0 = wpool.tile([128, C], BF16)
        nc.sync.dma_start(out=zT0, in_=zT_d[0:128])
        zT1 = wpool.tile([128, C], BF16)
        nc.sync.dma_start(out=zT1, in_=zT_d[128:256])
        fc1T0 = wpool.tile([128, 4 * C], BF16)
        nc.sync.dma_start(out=fc1T0, in_=fc1T_d[0:128])
        fc1T1 = wpool.tile([128, 4 * C], BF16)
        nc.sync.dma_start(out=fc1T1, in_=fc1T_d[128:256])
        fc2T = wpool.tile([128, 8, C], BF16)
        nc.sync.dma_start(out=fc2T, in_=fc2T_d[:].rearrange("(k p) m -> p k m", p=128))
        cb0 = wpool.tile([64, 1], F32)
        nc.sync.dma_start(out=cb0, in_=cb0_d[:])
        cb1 = wpool.tile([96, 1], F32)
        nc.sync.dma_start(out=cb1, in_=cb1_d[:])
        cb2 = wpool.tile([112, 1], F32)
        nc.sync.dma_start(out=cb2, in_=cb2_d[:])
        vecs = {}
        for nm, d in [("n1g", n1g_d), ("n1b", n1b_d), ("n2g", n2g_d),
                      ("n2b", n2b_d), ("gz", gz_d), ("bz", bz_d),
                      ("cascb", cascb_d), ("fc1b", fc1b_d), ("fc2b", fc2b_d)]:
            t = wpool.tile(list(d.shape), F32, name=f"v_{nm}")
            nc.sync.dma_start(out=t, in_=d[:])
            vecs[nm] = t
        ones_f = wpool.tile([128, 128], F32)
        nc.vector.memset(ones_f, 1.0)
        eps_t = wpool.tile([128, 1], F32)
        nc.vector.memset(eps_t, EPS)
        # GN1-fold helper vectors: per-conv input-channel gamma/beta at the
        # conv's partition base (zeros on recurrent-input rows), plus a mask
        # that is 1.0 on recurrent rows (their weight scale must be 1).
        g_c0 = wpool.tile([64, 1], F32)
        nc.sync.dma_start(out=g_c0, in_=n1g_d[64:128, 0:1])
        b_c0 = wpool.tile([64, 1], F32)
        nc.sync.dma_start(out=b_c0, in_=n1b_d[64:128, 0:1])
        m_c0 = wpool.tile([64, 1], F32)
        nc.vector.memset(m_c0, 0.0)
        g_c1 = wpool.tile([96, 1], F32)
        nc.vector.memset(g_c1, 0.0)
        nc.sync.dma_start(out=g_c1[0:64], in_=n1g_d[0:64, 1:2])
        b_c1 = wpool.tile([96, 1], F32)
        nc.vector.memset(b_c1, 0.0)
        nc.sync.dma_start(out=b_c1[0:64], in_=n1b_d[0:64, 1:2])
        m_c1 = wpool.tile([96, 1], F32)
        nc.vector.memset(m_c1, 0.0)
        nc.vector.memset(m_c1[64:96], 1.0)
        g_c2 = wpool.tile([112, 1], F32)
        nc.vector.memset(g_c2, 0.0)
        nc.sync.dma_start(out=g_c2[0:64], in_=n1g_d[64:128, 1:2])
        b_c2 = wpool.tile([112, 1], F32)
        nc.vector.memset(b_c2, 0.0)
        nc.sync.dma_start(out=b_c2[0:64], in_=n1b_d[64:128, 1:2])
        m_c2 = wpool.tile([112, 1], F32)
        nc.vector.memset(m_c2, 0.0)
        nc.vector.memset(m_c2[64:112], 1.0)
        conv_fold = [(c0T, g_c0, b_c0, m_c0, cb0, 64, 64),
                     (c1T, g_c1, b_c1, m_c1, cb1, 96, 96),
                     (c2T, g_c2, b_c2, m_c2, cb2, 112, 112)]

        def cols(n):
            return slice(n * CW, (n + 1) * CW)

        def gn_scale_bias(Xs, g_sl, b_sl):
            """GroupNorm(1,C) stats over two [128,HW] blocks -> per-channel
            (scale, bias) [128,1] f32 tiles for each block."""
            ps = pp1.tile([128, 3], F32, tag="convps", name="ps_stat")
            for k, X in enumerate(Xs):
                stats = vpool.tile([128, NCH, 6], F32, tag="stats", name="stats")
                for j in range(NCH):
                    nc.vector.bn_stats(out=stats[:, j, :], in_=X[:, cols(j)])
                mv = vpool.tile([128, 2], F32, tag="mv", name="mv")
                nc.vector.bn_aggr(out=mv, in_=stats)
                s3 = vpool.tile([128, 3], F32, tag="s3", name="s3")
                nc.vector.tensor_copy(out=s3[:, 0:2], in_=mv)
                nc.vector.tensor_mul(s3[:, 2:3], mv[:, 0:1], mv[:, 0:1])
                nc.tensor.matmul(out=ps, lhsT=ones_f, rhs=s3,
                                 start=(k == 0), stop=(k == 1))
            pssb = vpool.tile([128, 3], F32, tag="pssb", name="pssb")
            nc.vector.tensor_copy(out=pssb, in_=ps)
            mean_t = vpool.tile([128, 1], F32, tag="mean_t", name="mean_t")
            nc.vector.tensor_scalar_mul(mean_t, pssb[:, 0:1], 1.0 / C)
            msq = vpool.tile([128, 1], F32, tag="msq", name="msq")
            nc.vector.tensor_mul(msq, mean_t, mean_t)
            e2 = vpool.tile([128, 1], F32, tag="e2", name="e2")
            nc.vector.tensor_add(e2, pssb[:, 1:2], pssb[:, 2:3])
            var_t = vpool.tile([128, 1], F32, tag="var_t", name="var_t")
            nc.vector.tensor_scalar(out=var_t, in0=e2, scalar1=1.0 / C,
                                    scalar2=msq, op0=ALU.mult, op1=ALU.subtract)
            sd = vpool.tile([128, 1], F32, tag="sd", name="sd")
            nc.scalar.activation(out=sd, in_=var_t, func=AF.Sqrt, bias=eps_t)
            rstd = vpool.tile([128, 1], F32, tag="rstd", name="rstd")
            nc.vector.reciprocal(rstd, sd)
            nms = vpool.tile([128, 1], F32, tag="nms", name="nms")
            nc.vector.tensor_scalar(out=nms, in0=mean_t, scalar1=rstd,
                                    scalar2=-1.0, op0=ALU.mult, op1=ALU.mult)
            outs = []
            for k in range(2):
                sc_ = vpool.tile([128, 1], F32, tag=f"gsc{k}", name=f"gsc{k}")
                nc.vector.tensor_mul(sc_, g_sl[k], rstd)
                bi_ = vpool.tile([128, 1], F32, tag=f"gbi{k}", name=f"gbi{k}")
                nc.vector.scalar_tensor_tensor(out=bi_, in0=g_sl[k], scalar=nms,
                                               in1=b_sl[k], op0=ALU.mult,
                                               op1=ALU.add)
                outs.append((sc_, bi_))
            return outs, rstd, nms

        def shift_dma(dst, d0, src_t, s0, s1, shift, eng="dma"):
            """Copy src rows [s0:s1] into dst rows [d0:d0+n] with the axial
            shift (edge keeps original). eng='dve' only for pieces whose
            in/out partition bases satisfy the 0/32/64/96 alignment rule;
            DMA is exempt from that constraint."""
            n = s1 - s0
            d1 = d0 + n

            def cp(dsl, ssl):
                if eng == "dve":
                    nc.vector.tensor_copy(out=dst[d0:d1, dsl],
                                          in_=src_t[s0:s1, ssl])
                else:
                    nc.sync.dma_start(out=dst[d0:d1, dsl], in_=src_t[s0:s1, ssl])

            def cp_edge(d3, s3_):
                if eng == "dve":
                    nc.vector.tensor_copy(out=d3, in_=s3_)
                else:
                    nc.sync.dma_start(out=d3, in_=s3_)

            if shift == "N":
                cp(slice(0, HW), slice(0, HW))
            elif shift == "R":
                cp(slice(1, HW), slice(0, HW - 1))
                d3 = dst[d0:d1].rearrange("p (h w) -> p h w", w=W)
                s3_ = src_t[s0:s1].rearrange("p (h w) -> p h w", w=W)
                cp_edge(d3[:, :, 0:1], s3_[:, :, 0:1])
            elif shift == "L":
                cp(slice(0, HW - 1), slice(1, HW))
                d3 = dst[d0:d1].rearrange("p (h w) -> p h w", w=W)
                s3_ = src_t[s0:s1].rearrange("p (h w) -> p h w", w=W)
                cp_edge(d3[:, :, W - 1:W], s3_[:, :, W - 1:W])
            elif shift == "D":
                cp(slice(W, HW), slice(0, HW - W))
                cp(slice(0, W), slice(0, W))
            elif shift == "U":
                cp(slice(0, HW - W), slice(W, HW))
                cp(slice(HW - W, HW), slice(HW - W, HW))

        def aff_act(dst, src, sc, bi):
            # GN affine on ACT (idle during the cascade phase)
            nc.scalar.activation(out=dst, in_=src, func=AF.Identity,
                                 bias=bi, scale=sc)

        def gn_finish(ps, g_sl, b_sl, inv):
            """From psum [128,2] = [sum(x), sum(x^2)] broadcast over
            partitions, compute per-block (scale, bias)."""
            pssb = vpool.tile([128, 2], F32, tag="pssb", name="pssb")
            nc.vector.tensor_copy(out=pssb, in_=ps)
            mean_t = vpool.tile([128, 1], F32, tag="mean_t", name="mean_t")
            nc.vector.tensor_scalar_mul(mean_t, pssb[:, 0:1], inv)
            msq = vpool.tile([128, 1], F32, tag="msq", name="msq")
            nc.vector.tensor_mul(msq, mean_t, mean_t)
            var_t = vpool.tile([128, 1], F32, tag="var_t", name="var_t")
            nc.vector.tensor_scalar(out=var_t, in0=pssb[:, 1:2], scalar1=inv,
                                    scalar2=msq, op0=ALU.mult, op1=ALU.subtract)
            sd = vpool.tile([128, 1], F32, tag="sd", name="sd")
            nc.scalar.activation(out=sd, in_=var_t, func=AF.Sqrt, bias=eps_t)
            rstd = vpool.tile([128, 1], F32, tag="rstd", name="rstd")
            nc.vector.reciprocal(rstd, sd)
            nms = vpool.tile([128, 1], F32, tag="nms", name="nms")
            nc.vector.tensor_scalar(out=nms, in0=mean_t, scalar1=rstd,
                                    scalar2=-1.0, op0=ALU.mult, op1=ALU.mult)
            outs = []
            for k in range(2):
                sc_ = vpool.tile([128, 1], F32, tag=f"gsc{k}", name=f"gsc{k}")
                nc.vector.tensor_mul(sc_, g_sl[k], rstd)
                bi_ = vpool.tile([128, 1], F32, tag=f"gbi{k}", name=f"gbi{k}")
                nc.vector.scalar_tensor_tensor(out=bi_, in0=g_sl[k], scalar=nms,
                                               in1=b_sl[k], op0=ALU.mult,
                                               op1=ALU.add)
                outs.append((sc_, bi_))
            return outs, rstd, nms

        def gn_scale_bias_pool(Xs, g_sl, b_sl):
            """GroupNorm stats via GPSIMD (idle engine): per-channel sum and
            sum-of-squares, then cross-partition ones-matmul."""
            ps = pp1.tile([128, 2], F32, tag="convps", name="ps_stat2")
            for k, X in enumerate(Xs):
                r2 = vpool.tile([128, 2], F32, tag="r2", name="r2")
                scr = spool.tile([128, HW], BF16, tag="scr", name="scr")
                nc.gpsimd.tensor_scalar(out=scr, in0=X, scalar1=1.0,
                                        scalar2=None, op0=ALU.mult,
                                        op1=ALU.add, accum_out=r2[:, 0:1])
                scr2 = spool.tile([128, HW], BF16, tag="scr", name="scr2")
                nc.gpsimd.scalar_tensor_tensor(
                    out=scr2, in0=X, scalar=0.0, in1=X,
                    op0=ALU.add, op1=ALU.mult, accum_out=r2[:, 1:2])
                nc.tensor.matmul(out=ps, lhsT=ones_f, rhs=r2,
                                 start=(k == 0), stop=(k == 1))
            return gn_finish(ps, g_sl, b_sl, 1.0 / (C * HW))

        # ---------------- per-sample pipeline ----------------
        # stage A: load, GN1, shifts, cascade convs, casc GN+conv+residual
        # stage B: GN2, fc1+gelu+fc2, output
        # Every engine executes its instruction stream in (priority ~)
        # emission order, so coarse per-sample emission serializes samples.
        # Emit in fine-grained interleave: parts of B(b) alternate with
        # parts of A(b+1), letting b+1's (DVE-heavy) cascade fill the gaps
        # of b's (PE/ACT-heavy) fc phase on every engine.
        def stage_a_parts(b, st):
            def p0():
                st["XA"] = xpool.tile([128, HW], F32, tag="XA", name="XA")
                st["XB"] = xpool.tile([128, HW], F32, tag="XB", name="XB")
                nc.sync.dma_start(out=st["XA"], in_=x_d[b, 0:128, :])
                nc.scalar.dma_start(out=st["XB"], in_=x_d[b, 128:256, :])
                st["gn1"], st["rstd1"], st["nms1"] = gn_scale_bias(
                    [st["XA"], st["XB"]],
                    [vecs["n1g"][:, 0:1], vecs["n1g"][:, 1:2]],
                    [vecs["n1b"][:, 0:1], vecs["n1b"][:, 1:2]])
                # fold GN1 into conv weights: scaled weights + bias vectors
                st["cTs"], st["cbf"] = [], []
                for i, (cT, g_c, b_c, m_c, cb_c, Ki, Mi) in enumerate(conv_fold):
                    scv = vpool.tile([Ki, 1], F32, tag=f"scv{i}", name=f"scv{i}")
                    nc.vector.scalar_tensor_tensor(
                        out=scv, in0=g_c, scalar=st["rstd1"][0:Ki], in1=m_c,
                        op0=ALU.mult, op1=ALU.add)
                    bvv = vpool.tile([Ki, 1], F32, tag=f"bvv{i}", name=f"bvv{i}")
                    nc.vector.scalar_tensor_tensor(
                        out=bvv, in0=g_c, scalar=st["nms1"][0:Ki], in1=b_c,
                        op0=ALU.mult, op1=ALU.add)
                    bvvb = vpool.tile([Ki, 1], BF16, tag=f"bvvb{i}",
                                      name=f"bvvb{i}")
                    nc.vector.tensor_copy(out=bvvb, in_=bvv)
                    cTs = wspool.tile([Ki, Mi], BF16, tag=f"cts{i}",
                                      name=f"cts{i}")
                    nc.vector.tensor_scalar_mul(cTs, cT, scv)
                    psbf = pp1.tile([Mi, 1], F32, tag="convps", name="psbfc")
                    nc.tensor.matmul(out=psbf, lhsT=cT, rhs=bvvb,
                                     start=True, stop=True)
                    bfc = wspool.tile([Mi, 1], F32, tag=f"bfc{i}",
                                      name=f"bfc{i}")
                    nc.scalar.activation(out=bfc, in_=psbf, func=AF.Identity,
                                         bias=cb_c)
                    st["cTs"].append(cTs)
                    st["cbf"].append(bfc)

            def p1():
                XA, XB = st["XA"], st["XB"]
                (scA, biA), (scB, biB) = st["gn1"]
                S1 = st["S1"] = spool.tile([64, HW], BF16, tag="sh_d", name="S1")
                S2 = st["S2"] = spool.tile([96, HW], BF16, tag="sh_e", name="S2")
                S3 = st["S3"] = spool.tile([112, HW], BF16, tag="sh_f", name="S3")
                st["Z0"] = zpool.tile([128, HW], BF16, tag="Z0", name="Z0")
                Z1 = st["Z1"] = zpool.tile([128, HW], BF16, tag="Z1", name="Z1")
                Y1 = spool.tile([64, HW], BF16, tag="sh_a", name="Y1")
                Y2 = spool.tile([64, HW], BF16, tag="sh_b", name="Y2")
                Y3 = spool.tile([64, HW], BF16, tag="sh_c", name="Y3")
                # xs0 needs GN1 applied (it feeds the cascade GN directly);
                # xs1/2/3 are consumed by convs whose weights absorb GN1,
                # so they are plain bf16 casts with no stats dependency.
                aff_act(Z1[64:128], XA[0:64], scA[0:64], biA[0:64])   # xs0
                nc.scalar.copy(out=Y1, in_=XA[64:128])                # xs1 raw
                nc.vector.tensor_copy(out=Y2, in_=XB[0:64])           # xs2 raw
                nc.vector.tensor_copy(out=Y3, in_=XB[64:128])         # xs3 raw
                shift_dma(S1, 0, Y1, 0, 16, "R", eng="dve")
                shift_dma(S1, 16, Y1, 16, 32, "L")
                shift_dma(S1, 32, Y1, 32, 48, "D", eng="dve")
                shift_dma(S1, 48, Y1, 48, 64, "U")
                shift_dma(S2, 0, Y2, 0, 24, "R", eng="dve")
                shift_dma(S2, 24, Y2, 24, 48, "L")
                shift_dma(S2, 48, Y2, 48, 64, "D")
                shift_dma(S3, 0, Y3, 0, 28, "R", eng="dve")
                shift_dma(S3, 28, Y3, 28, 56, "L")
                shift_dma(S3, 56, Y3, 56, 64, "D")

            def p2():
                # conv c0: [64]->[64] = [x1(32) | o1(32)]; o1 direct to Z0
                S1, S2, Z0 = st["S1"], st["S2"], st["Z0"]
                X1 = spool.tile([32, HW], BF16, tag="sh_a", name="X1")
                for n in range(NCH):
                    ps0 = pp1.tile([64, CW], F32, tag="convps", name="ps0")
                    nc.tensor.matmul(out=ps0, lhsT=st["cTs"][0], rhs=S1[:, cols(n)],
                                     start=True, stop=True)
                    nc.vector.tensor_scalar_add(out=X1[:, cols(n)],
                                                in0=ps0[0:32],
                                                scalar1=st["cbf"][0][0:32])
                    nc.vector.tensor_scalar_add(out=Z0[0:32, cols(n)],
                                                in0=ps0[32:64],
                                                scalar1=st["cbf"][0][32:64])
                shift_dma(S2, 64, X1, 0, 8, "D", eng="dve")   # x1 q2-part
                shift_dma(S2, 72, X1, 8, 32, "U")             # x1 q3

            def p3():
                # conv c1: [96]->[96] = [x1b(48) | o2(48)]
                S2, S3, Z0 = st["S2"], st["S3"], st["Z0"]
                O2 = spool.tile([96, HW], BF16, tag="sh_d", name="O2")
                for n in range(NCH):
                    ps1 = pp1.tile([96, CW], F32, tag="convps", name="ps1")
                    nc.tensor.matmul(out=ps1, lhsT=st["cTs"][1], rhs=S2[:, cols(n)],
                                     start=True, stop=True)
                    nc.vector.tensor_scalar_add(out=O2[:, cols(n)], in0=ps1,
                                                scalar1=st["cbf"][1])
                shift_dma(S3, 64, O2, 0, 20, "D", eng="dve")  # x1b q2-part
                shift_dma(S3, 84, O2, 20, 48, "U")            # x1b q3
                shift_dma(Z0, 32, O2, 48, 96, "N")            # o2

            def p4():
                # conv c2: [112]->[112], rows [o3b(64) | o3a(48)]
                S3, Z0, Z1 = st["S3"], st["Z0"], st["Z1"]
                O3 = spool.tile([48, HW], BF16, tag="sh_e", name="O3")
                for n in range(NCH):
                    ps2 = pp1.tile([112, CW], F32, tag="convps", name="ps2")
                    nc.tensor.matmul(out=ps2, lhsT=st["cTs"][2], rhs=S3[:, cols(n)],
                                     start=True, stop=True)
                    nc.vector.tensor_scalar_add(out=Z1[0:64, cols(n)],
                                                in0=ps2[0:64],
                                                scalar1=st["cbf"][2][0:64])
                    nc.vector.tensor_scalar_add(out=O3[:, cols(n)],
                                                in0=ps2[64:112],
                                                scalar1=st["cbf"][2][64:112])
                shift_dma(Z0, 80, O3, 0, 48, "N")             # o3a

            def p5():
                # cascade GN (folded into weights): stats + scaled weights
                Z0, Z1 = st["Z0"], st["Z1"]
                gnz, _, _ = gn_scale_bias(
                    [Z0, Z1],
                    [vecs["gz"][:, 0:1], vecs["gz"][:, 1:2]],
                    [vecs["bz"][:, 0:1], vecs["bz"][:, 1:2]])
                Wzs, bzbf = [], []
                for k, (zT_k, (sc_k, bi_k)) in enumerate(zip([zT0, zT1], gnz)):
                    w_s = wspool.tile([128, C], BF16, tag=f"wzs{k}",
                                      name=f"wzs{k}")
                    nc.vector.tensor_scalar_mul(w_s, zT_k, sc_k)
                    Wzs.append(w_s)
                    bb = wspool.tile([128, 1], BF16, tag=f"bzbf{k}",
                                     name=f"bzbf{k}")
                    nc.vector.tensor_copy(out=bb, in_=bi_k)
                    bzbf.append(bb)
                st["Wzs"] = Wzs
                bfs = []
                for m in range(2):
                    psbf = pp1.tile([128, 1], F32, tag="convps", name="psbf")
                    for k in range(2):
                        nc.tensor.matmul(
                            out=psbf,
                            lhsT=[zT0, zT1][k][:, 128 * m:128 * m + 128],
                            rhs=bzbf[k], start=(k == 0), stop=(k == 1))
                    bf_sb = wspool.tile([128, 1], F32, tag=f"bfsb{m}",
                                        name=f"bfsb{m}")
                    nc.scalar.activation(out=bf_sb, in_=psbf, func=AF.Identity,
                                         bias=vecs["cascb"][:, m:m + 1])
                    bfs.append(bf_sb)
                st["bfs"] = bfs

            def casc_chunks(lo, hi):
                def run():
                    Z0, Z1 = st["Z0"], st["Z1"]
                    Wzs, bfs = st["Wzs"], st["bfs"]
                    for n in range(lo, hi):
                        for m, X_m in enumerate([st["XA"], st["XB"]]):
                            psz = ppzo.tile([128, CW], F32, tag="pszo",
                                            name="psz")
                            nc.tensor.matmul(
                                out=psz, lhsT=Wzs[0][:, 128 * m:128 * m + 128],
                                rhs=Z0[:, cols(n)], start=True, stop=False)
                            nc.tensor.matmul(
                                out=psz, lhsT=Wzs[1][:, 128 * m:128 * m + 128],
                                rhs=Z1[:, cols(n)], start=False, stop=True)
                            # x_new = x + psz + bf  (in place into X_m)
                            nc.vector.scalar_tensor_tensor(
                                out=X_m[:, cols(n)], in0=psz, scalar=bfs[m],
                                in1=X_m[:, cols(n)], op0=ALU.add, op1=ALU.add)
                return run

            return [p0, p1, p2, p3, p4, p5,
                    casc_chunks(0, 2), casc_chunks(2, 5), casc_chunks(5, NCH)]

        def stage_b_parts(b, st):
            def q0():
                XA, XB = st["XA"], st["XB"]
                gn2, _, _ = gn_scale_bias(
                    [XA, XB],
                    [vecs["n2g"][:, 0:1], vecs["n2g"][:, 1:2]],
                    [vecs["n2b"][:, 0:1], vecs["n2b"][:, 1:2]])
                M0 = st["M0"] = zpool.tile([128, HW], BF16, tag="M0", name="M0")
                M1 = st["M1"] = zpool.tile([128, HW], BF16, tag="M1", name="M1")
                # GN2 folded into fc1 weights: M tiles are raw bf16 casts
                nc.vector.tensor_copy(out=M0, in_=XA)
                nc.scalar.copy(out=M1, in_=XB)
                f1s, bv2 = [], []
                for k, fT in enumerate([fc1T0, fc1T1]):
                    sck, bik = gn2[k]
                    fs = wspool.tile([128, 4 * C], BF16, tag=f"f1s{k}",
                                     name=f"f1s{k}")
                    nc.vector.tensor_scalar_mul(fs, fT, sck)
                    f1s.append(fs)
                    bb = vpool.tile([128, 1], BF16, tag=f"bv2{k}",
                                    name=f"bv2{k}")
                    nc.vector.tensor_copy(out=bb, in_=bik)
                    bv2.append(bb)
                psb1 = pp1.tile([128, 8], F32, tag="convps", name="psb1")
                for mo in range(8):
                    for k in range(2):
                        nc.tensor.matmul(
                            out=psb1[:, mo:mo + 1],
                            lhsT=[fc1T0, fc1T1][k][:, 128 * mo:128 * mo + 128],
                            rhs=bv2[k], start=(k == 0), stop=(k == 1))
                gb1 = vpool.tile([128, 8], F32, tag="gb1", name="gb1")
                nc.vector.tensor_add(gb1, psb1, vecs["fc1b"])
                st["f1s"], st["gb1"] = f1s, gb1

            def fc_pair(n0):
                def run():
                    XA, XB, M0, M1 = st["XA"], st["XB"], st["M0"], st["M1"]
                    npair = min(2, NCH - n0)
                    pw = npair * CW
                    h = hpool.tile([128, 8, 2, CW], BF16, tag="h", name="h")
                    for mo in range(8):
                        psh = pph.tile([128, 2, 512], F32, tag="psh",
                                       name="psh")
                        for j in range(npair):
                            nc.tensor.matmul(
                                out=psh[:, j, 0:CW],
                                lhsT=st["f1s"][0][:, 128 * mo:128 * mo + 128],
                                rhs=M0[:, cols(n0 + j)], start=True, stop=False)
                            nc.tensor.matmul(
                                out=psh[:, j, 0:CW],
                                lhsT=st["f1s"][1][:, 128 * mo:128 * mo + 128],
                                rhs=M1[:, cols(n0 + j)], start=False, stop=True)
                        nc.scalar.activation(out=h[:, mo, 0:npair, :],
                                             in_=psh[:, 0:npair, 0:CW],
                                             func=gelu_fn,
                                             bias=st["gb1"][:, mo:mo + 1])
                    for j in range(npair):
                        n = n0 + j
                        for m, X_m in enumerate([XA, XB]):
                            pso = ppzo.tile([128, CW], F32, tag="pszo",
                                            name="pso")
                            for k in range(8):
                                nc.tensor.matmul(
                                    out=pso,
                                    lhsT=fc2T[:, k, 128 * m:128 * m + 128],
                                    rhs=h[:, k, j, :],
                                    start=(k == 0), stop=(k == 7))
                            # out = x_new + pso + fc2_b, in place into X_m
                            nc.vector.scalar_tensor_tensor(
                                out=X_m[:, cols(n)], in0=pso,
                                scalar=vecs["fc2b"][:, m:m + 1],
                                in1=X_m[:, cols(n)], op0=ALU.add, op1=ALU.add)
                    pc = slice(n0 * CW, (n0 + npair) * CW)
                    nc.sync.dma_start(out=out_d[b, 0:128, pc],
                                      in_=st["XA"][:, pc])
                    nc.scalar.dma_start(out=out_d[b, 128:256, pc],
                                        in_=st["XB"][:, pc])
                return run

            return [q0] + [fc_pair(n0) for n0 in range(0, NCH, 2)]

        states = [dict() for _ in range(bl)]
        for part in stage_a_parts(0, states[0]):
            part()
        for b in range(bl):
            bq = stage_b_parts(b, states[b])
            ap = stage_a_parts(b + 1, states[b + 1]) if b + 1 < bl else []
            # round-robin interleave
            i = j = 0
            while i < len(bq) or j < len(ap):
                if i < len(bq):
                    bq[i](); i += 1
                if j < len(ap):
                    ap[j](); j += 1

    nc.compile()
    return nc


def prep_inputs(c0_w, c0_b, c1_w, c1_b, c2_w, c2_b, casc_w, casc_b, casc_g,
                casc_beta, n1_g, n1_b, n2_g, n2_b, fc1_w, fc1_b, fc2_w, fc2_b):
    """Host-side weight prep: transposes, bf16 casts, shuffle permutation."""
    bf = lambda a: np.ascontiguousarray(a).astype(ml_dtypes.bfloat16)
    f32 = lambda a: np.ascontiguousarray(a, dtype=np.float32)

    # channel shuffle (groups=2): shuffled[2i+j] = z[j*128+i]
    zc = np.arange(C)
    sc_of_zc = 2 * (zc % 128) + (zc // 128)

    def blocks(v):  # [256] -> [128, 2]
        return f32(np.stack([v[0:128], v[128:256]], axis=1))

    # c2 output rows reordered to [o3b(ch 48:112) | o3a(ch 0:48)] so o3b
    # drains partition-aligned straight into Z1[0:64]
    c2perm = np.concatenate([np.arange(48, 112), np.arange(0, 48)])
    return {
        "c0T": bf(c0_w.T), "c1T": bf(c1_w.T), "c2T": bf(c2_w[c2perm].T),
        "zT": bf(casc_w[:, sc_of_zc].T),
        "fc1T": bf(fc1_w.T), "fc2T": bf(fc2_w.T),
        "cb0": f32(c0_b[:, None]), "cb1": f32(c1_b[:, None]),
        "cb2": f32(c2_b[c2perm][:, None]),
        "n1g": blocks(n1_g), "n1b": blocks(n1_b),
        "n2g": blocks(n2_g), "n2b": blocks(n2_b),
        "gz": blocks(casc_g[sc_of_zc]), "bz": blocks(casc_beta[sc_of_zc]),
        "cascb": blocks(casc_b),
        "fc1b": f32(fc1_b.reshape(8, 128).T),
        "fc2b": blocks(fc2_b),
    }


_NC_CACHE = {}


def kernel(**inputs):
    x = np.asarray(inputs["x"], dtype=np.float32)
    wmap = prep_inputs(**{k: np.asarray(v) for k, v in inputs.items()
                          if k != "x"})

    key = "main"
    if key not in _NC_CACHE:
        _NC_CACHE[key] = build()
    nc = _NC_CACHE[key]

    xr = x.reshape(B, C, HW)
    in_maps = []
    for c in range(NCORES):
        m = dict(wmap)
        m["x"] = np.ascontiguousarray(xr[c * BL:(c + 1) * BL])
        in_maps.append(m)

    res = run_bass_kernel_spmd(nc, in_maps, core_ids=list(range(NCORES)))
    out = np.concatenate([r["out"] for r in res.results], axis=0)
    return out.reshape(B, C, H, W).astype(np.float32)


if __name__ == "__main__":
    rng = np.random.default_rng(0)
    fake = {
        "x": rng.standard_normal((B, C, H, W), dtype=np.float32),
        "c0_w": rng.standard_normal((64, 64), dtype=np.float32) * 0.02,
        "c0_b": np.zeros(64, np.float32),
        "c1_w": rng.standard_normal((96, 96), dtype=np.float32) * 0.02,
        "c1_b": np.zeros(96, np.float32),
        "c2_w": rng.standard_normal((112, 112), dtype=np.float32) * 0.02,
        "c2_b": np.zeros(112, np.float32),
        "casc_w": rng.standard_normal((C, C), dtype=np.float32) * 0.02,
        "casc_b": np.zeros(C, np.float32),
        "casc_g": np.ones(C, np.float32), "casc_beta": np.zeros(C, np.float32),
        "n1_g": np.ones(C, np.float32), "n1_b": np.zeros(C, np.float32),
        "n2_g": np.ones(C, np.float32), "n2_b": np.zeros(C, np.float32),
        "fc1_w": rng.standard_normal((4 * C, C), dtype=np.float32) * 0.02,
        "fc1_b": np.zeros(4 * C, np.float32),
        "fc2_w": rng.standard_normal((C, 4 * C), dtype=np.float32) * 0.02,
        "fc2_b": np.zeros(C, np.float32),
    }
    out = kernel(**fake)
    print("kernel ran, out shape", out.shape)
